# revision 1
# baseline (speedup 1.0000x reference)
"""Trainium2 Bass kernel for nn_Att_H (rank-1 attention MLP), 8-core data parallel.

Math (per sample b):
  h = silu(x @ W_in.T + b_in)
  Q,K,V = silu(h @ A*.T + B*)
  scores[i,j] = Q[i]*K[j];  attn = softmax_j;  ctx = silu(attn @ V)
  y = silu(ctx @ W_out.T + b_out);  out = quad-form tail on y.

Key trick: softmax_j(Q_i*K_j) == softmax_j(Q_i*(K_j - Kmax)).  With
Ktil = K - Kmax, shifted scores are <= ~13 (exp is fp32-safe, no NaN since
the j=argmax term contributes exp(Q_i*0)=1), and the per-row shift
exp(-Q_i*Kmax) cancels in num/den.  So:
  E_T[j, i] = exp(Ktil_j * Q_i)           (ACT exp: input = Q broadcast
                                           across partitions, per-partition
                                           scale = Ktil_j)
  [num_i; den_i] = [V;1].T @ E_T          (PE matmul, contraction over j)
  ctx_i = silu(num_i / den_i)

All silus are computed as x*sigmoid(x) = x/(1+exp(-x)) using only the Exp
activation so a single ACT table set is ever loaded.
"""

import sys
import numpy as np

for _p in ("/opt/trn_rl_repo", "/opt/trn_rl_repo/concourse"):
    if _p not in sys.path:
        sys.path.append(_p)

B_GLOBAL = 512
N_CORES = 8
B = B_GLOBAL // N_CORES  # 64 samples per core
IN = 128
H = 512
O = 25
NSEG = H // 128  # 4

_cache = {}

# "a": per-(sample, jseg) ACT exp with per-partition scale (fewer DVE ops,
#      more ACT instruction overhead).
# "b": DVE materializes S = Ktil_j * Q_i into a big fp32 buffer, ACT exps it
#      in [128, 4096] chunks (2 samples per instruction).
VARIANT = "b"
EXP_GROUP = 2  # samples per ACT exp instruction in variant b


def _build_nc():
    from contextlib import ExitStack

    import concourse.bass as bass
    import concourse.tile as tile
    from concourse import bacc, mybir

    f32 = mybir.dt.float32
    bf16 = mybir.dt.bfloat16
    EXP = mybir.ActivationFunctionType.Exp
    SIG = mybir.ActivationFunctionType.Sigmoid
    AX = mybir.AxisListType.X

    nc = bacc.Bacc()
    x_d = nc.declare_dram_parameter("xT", [IN, B], f32, False)
    w_inT_d = nc.declare_dram_parameter("w_inT", [IN, H], f32, False)
    aT_d = [nc.declare_dram_parameter(f"a{m}T", [H, H], f32, False) for m in "qkv"]
    b_in_d = nc.declare_dram_parameter("b_in_bc", [B, H], f32, False)
    bb_d = [nc.declare_dram_parameter(f"b{m}_bc", [B, H], f32, False) for m in "qkv"]
    w_outT_d = nc.declare_dram_parameter("w_outT", [H, O], f32, False)
    b_out_d = nc.declare_dram_parameter("b_out_bc", [B, O], f32, False)
    eye_d = nc.declare_dram_parameter("eye64", [B, B], f32, False)
    out_d = nc.declare_dram_parameter("out", [B, 1], f32, True)
    q_dram = nc.dram_tensor("q_scratch", [B, H], f32)

    with tile.TileContext(nc) as tc, ExitStack() as ctx:
        const_pool = ctx.enter_context(tc.tile_pool(name="const", bufs=1))
        big_pool = ctx.enter_context(tc.tile_pool(name="big", bufs=1))
        work_pool = ctx.enter_context(tc.tile_pool(name="work", bufs=2))
        qb_pool = ctx.enter_context(tc.tile_pool(name="qb", bufs=6))
        sc_pool = ctx.enter_context(tc.tile_pool(name="sc", bufs=2))
        e_pool = ctx.enter_context(tc.tile_pool(name="et", bufs=2))
        ea_pool = ctx.enter_context(tc.tile_pool(name="eta", bufs=1))
        psum_mm = ctx.enter_context(tc.tile_pool(name="psmm", bufs=1, space="PSUM"))
        psum_tp = ctx.enter_context(tc.tile_pool(name="pstp", bufs=2, space="PSUM"))
        psum_nd = ctx.enter_context(tc.tile_pool(name="psnd", bufs=2, space="PSUM"))

        # ---- load params ----
        xT_sb = const_pool.tile([IN, B], f32)
        nc.sync.dma_start(xT_sb[:], x_d[:])
        w_inT_sb = const_pool.tile([IN, H], f32)
        nc.sync.dma_start(w_inT_sb[:], w_inT_d[:])
        aT_sb = []
        for mi, d in enumerate(aT_d):
            t = big_pool.tile([128, NSEG, H], f32, tag=f"aT{mi}")
            nc.sync.dma_start(t[:], d[:].rearrange("(s p) i -> p s i", p=128))
            aT_sb.append(t)
        b_in_sb = const_pool.tile([B, H], f32)
        nc.sync.dma_start(b_in_sb[:], b_in_d[:])
        bb_sb = []
        for mi, d in enumerate(bb_d):
            t = const_pool.tile([B, H], f32, tag=f"bb{mi}")
            nc.sync.dma_start(t[:], d[:])
            bb_sb.append(t)
        w_outT_sb = const_pool.tile([128, NSEG, O], f32)
        nc.sync.dma_start(w_outT_sb[:], w_outT_d[:].rearrange("(s p) o -> p s o", p=128))
        b_out_sb = const_pool.tile([B, O], f32)
        nc.sync.dma_start(b_out_sb[:], b_out_d[:])
        eye_sb = const_pool.tile([B, B], f32)
        nc.sync.dma_start(eye_sb[:], eye_d[:])
        ones_sb = const_pool.tile([1, 128], f32)
        nc.vector.memset(ones_sb[:], 1.0)
        # VO[:, s, b, :] = [V_T, 1.0] (f32r moving pairs, pair contiguous)
        f32r = mybir.dt.float32r
        vo_sb = const_pool.tile([128, NSEG, B, 2], f32r)
        ones128 = const_pool.tile([128, B], f32, tag="ones128")
        nc.vector.memset(ones128[:], 1.0)
        for s in range(NSEG):
            nc.vector.tensor_copy(vo_sb[:, s, :, 1], ones128[:])

        def silu_inplace(pre_ap, out_tile):
            """out = silu(pre) = pre * sigmoid(pre)."""
            shp = list(pre_ap.shape)
            e = work_pool.tile(shp, f32, tag="silu_e")
            nc.scalar.activation(e[:], pre_ap, SIG, bias=0.0, scale=1.0)
            nc.vector.tensor_mul(out_tile[:], pre_ap, e[:])

        def transpose_to_sb(src_ap, dst_ap, dtype=f32):
            """[64, 128] SBUF slice -> [128, 64] SBUF (via PE transpose + copy)."""
            pt = psum_tp.tile([128, B], f32, tag="tp")
            nc.tensor.transpose(pt[:], src_ap, eye_sb[:])
            nc.vector.tensor_copy(dst_ap, pt[:])

        # ---- h = silu(x @ W_in.T + b_in) ----
        h_ps = psum_mm.tile([B, H], f32, tag="mma")
        nc.tensor.matmul(h_ps[:], lhsT=xT_sb[:], rhs=w_inT_sb[:], start=True, stop=True)
        h_pre = work_pool.tile([B, H], f32, tag="hpre")
        nc.vector.tensor_add(h_pre[:], h_ps[:], b_in_sb[:])
        h_sb = const_pool.tile([B, H], f32)
        silu_inplace(h_pre[:], h_sb)

        # ---- h_T (f32r for the QKV matmuls) ----
        hT_sb = const_pool.tile([128, NSEG, B], f32)
        for s in range(NSEG):
            transpose_to_sb(h_sb[:, 128 * s : 128 * (s + 1)], hT_sb[:, s, :])

        # ---- K, Q, V (K first so the attention prep starts earliest) ----
        qkv_sb = [None, None, None]
        for m in (1, 0, 2):
            ps = psum_mm.tile([B, H], f32, tag="mma" if m % 2 else "mmb")
            for s in range(NSEG):
                nc.tensor.matmul(
                    ps[:],
                    lhsT=hT_sb[:, s, :],
                    rhs=aT_sb[m][:, s, :],
                    start=(s == 0),
                    stop=(s == NSEG - 1),
                )
            pre = work_pool.tile([B, H], f32, tag="qkvpre")
            nc.vector.tensor_add(pre[:], ps[:], bb_sb[m][:])
            t = const_pool.tile([B, H], f32, tag=f"qkv{m}")
            silu_inplace(pre[:], t)
            qkv_sb[m] = t
            if m == 1:
                k_sb = t
                kmax = work_pool.tile([B, 1], f32, tag="kmax")
                nc.vector.tensor_reduce(
                    kmax[:], k_sb[:], axis=AX, op=mybir.AluOpType.max
                )
                ktil = const_pool.tile([B, H], f32)
                nc.vector.tensor_scalar_sub(ktil[:], k_sb[:], kmax[:])
                ktilT_sb = const_pool.tile([128, NSEG, B], f32)
                for s in range(NSEG):
                    transpose_to_sb(
                        ktil[:, 128 * s : 128 * (s + 1)], ktilT_sb[:, s, :]
                    )
            elif m == 0:
                nc.sync.dma_start(q_dram[:], t[:])
        q_sb, k_sb, v_sb = qkv_sb

        for s in range(NSEG):
            pt = psum_tp.tile([128, B], f32, tag="tp")
            nc.tensor.transpose(pt[:], v_sb[:, 128 * s : 128 * (s + 1)], eye_sb[:])
            nc.vector.tensor_copy(vo_sb[:, s, :, 0], pt[:])

        # ---- attention: per-sample rank-1 scores -> exp -> [V;1] matvec ----
        # Per sample: 4 accumulating matmuls with stationary = [V;1] pair
        # [128, 2] (f32r) and moving = E_T j-seg [128, 512] (f32r, 1 cyc/row)
        # -> nd [2, 512] in its own PSUM bank at partition base 0.  Drained
        # via partition-preserving [2,512] copies (split ACT/DVE) into a
        # [2, DR, 512] staging tile, then repacked to [DR, 512] rows by
        # SBUF->SBUF DMA (DMA can move across partitions; engines cannot).
        num_sb = const_pool.tile([B, H], f32, tag="num")
        den_sb = const_pool.tile([B, H], f32, tag="den")
        G = EXP_GROUP
        DR = 8
        stage = None
        for g in range(B // G):
            b0 = G * g
            # one DMA broadcasts G consecutive Q rows across all partitions
            qb2 = qb_pool.tile([128, G, H], f32, tag="qb")
            nc.sync.dma_start(
                qb2[:],
                q_dram[b0 : b0 + G, :].unsqueeze(0).broadcast_to((128, G, H)),
            )
            a_mode = {b0} if g % 8 == 0 else set()
            sc = sc_pool.tile([128, G, NSEG, H], f32, tag="sc")
            ets = {}
            for r in range(G):
                b = b0 + r
                if b in a_mode:
                    ea = ea_pool.tile([128, NSEG, H], f32r, tag="eta")
                    for s in range(NSEG):
                        nc.scalar.activation(
                            ea[:, s, :], qb2[:, r, :], EXP, bias=0.0,
                            scale=ktilT_sb[:, s, b : b + 1],
                        )
                    ets[r] = ea
                else:
                    for s in range(NSEG):
                        nc.vector.tensor_scalar_mul(
                            sc[:, r, s, :], qb2[:, r, :], ktilT_sb[:, s, b : b + 1]
                        )
            et = e_pool.tile([128, G, NSEG, H], f32r, tag="et")
            bmode = [r for r in range(G) if b0 + r not in a_mode]
            if len(bmode) == G:
                nc.scalar.activation(et[:], sc[:], EXP, bias=0.0, scale=1.0)
            else:
                for r in bmode:
                    nc.scalar.activation(
                        et[:, r, :, :], sc[:, r, :, :], EXP, bias=0.0, scale=1.0
                    )
            ndbs = {}
            for r in range(G):
                b = b0 + r
                if b % 2 == 0:
                    ndb = psum_nd.tile([2, 2, H], f32, tag="nd")
                    ndbs[b] = ndb
                else:
                    ndb = ndbs[b - 1]
                esrc = ets[r][:] if r in ets else et[:, r, :, :]
                for s in range(NSEG):
                    nc.tensor.matmul(
                        ndb[:, b % 2, :],
                        lhsT=vo_sb[:, s, b, :],
                        rhs=esrc[:, s, :],
                        start=(s == 0),
                        stop=(s == NSEG - 1),
                    )
                if b % DR == 0:
                    stage = work_pool.tile([2, DR, H], f32, tag="stage")
                if b % 2 == 1:
                    if (b // 2) % 2 == 0:
                        nc.scalar.copy(stage[:, b % DR - 1 : b % DR + 1, :], ndb[:])
                    else:
                        nc.vector.tensor_copy(
                            stage[:, b % DR - 1 : b % DR + 1, :], ndb[:]
                        )
                if b % DR == DR - 1:
                    r0 = b - (DR - 1)
                    nc.sync.dma_start(num_sb[r0 : r0 + DR, :], stage[0:1, :, :])
                    nc.sync.dma_start(den_sb[r0 : r0 + DR, :], stage[1:2, :, :])

        # ---- ctx = silu(num/den) ----
        nc.vector.reciprocal(den_sb[:], den_sb[:])
        ctx0 = work_pool.tile([B, H], f32, tag="ctx0")
        nc.vector.tensor_mul(ctx0[:], num_sb[:], den_sb[:])
        ctx_sb = const_pool.tile([B, H], f32, tag="ctx")
        silu_inplace(ctx0[:], ctx_sb)

        # ---- ctx_T ----
        ctxT_sb = work_pool.tile([128, NSEG, B], f32, tag="ctxT")
        for s in range(NSEG):
            transpose_to_sb(ctx_sb[:, 128 * s : 128 * (s + 1)], ctxT_sb[:, s, :])

        # ---- y = silu(ctx @ W_out.T + b_out) ----
        y_ps = psum_mm.tile([B, O], f32, tag="mmb")
        for s in range(NSEG):
            nc.tensor.matmul(
                y_ps[:],
                lhsT=ctxT_sb[:, s, :],
                rhs=w_outT_sb[:, s, :],
                start=(s == 0),
                stop=(s == NSEG - 1),
            )
        y_pre = work_pool.tile([B, O], f32, tag="ypre")
        nc.vector.tensor_add(y_pre[:], y_ps[:], b_out_sb[:])
        y_sb = work_pool.tile([B, O], f32, tag="y")
        silu_inplace(y_pre[:], y_sb)

        # ---- tail: block-diag quadratic form ----
        y2 = work_pool.tile([B, O], f32, tag="y2")
        nc.vector.tensor_mul(y2[:], y_sb[:], y_sb[:])
        m_sb = work_pool.tile([B, 5], f32, tag="m5")
        nc.vector.tensor_reduce(
            m_sb[:], y2[:].rearrange("p (a b) -> p a b", b=5), axis=AX,
            op=mybir.AluOpType.add,
        )
        # p = [q0^2+q1^2, q2^2+q3^2]; c = [q0*q2, q1*q3]
        p2 = work_pool.tile([B, 2], f32, tag="p2")
        nc.vector.tensor_add(p2[:], y2[:, 0:3:2], y2[:, 1:4:2])
        c2 = work_pool.tile([B, 2], f32, tag="c2")
        nc.vector.tensor_mul(c2[:], y_sb[:, 0:2], y_sb[:, 2:4])
        cc = work_pool.tile([B, 1], f32, tag="cc")
        nc.vector.tensor_add(cc[:], c2[:, 0:1], c2[:, 1:2])
        mm12 = work_pool.tile([B, 1], f32, tag="mm12")
        nc.vector.tensor_add(mm12[:], m_sb[:, 1:2], m_sb[:, 2:3])
        mp = work_pool.tile([B, 2], f32, tag="mp")
        nc.vector.tensor_mul(mp[:], m_sb[:, 0:4:3], p2[:])
        acc = work_pool.tile([B, 1], f32, tag="acc")
        nc.vector.tensor_add(acc[:], mp[:, 0:1], mp[:, 1:2])
        acc2 = work_pool.tile([B, 1], f32, tag="acc2")
        nc.vector.tensor_mul(acc2[:], mm12[:], cc[:])
        acc3 = work_pool.tile([B, 1], f32, tag="acc3")
        nc.vector.tensor_add(acc3[:], acc[:], acc2[:])
        res = work_pool.tile([B, 1], f32, tag="res")
        nc.vector.tensor_add(res[:], acc3[:], m_sb[:, 4:5])
        nc.sync.dma_start(out_d[:], res[:])

    nc.finalize()
    return nc


def _host_inputs(x, W_in, b_in, Aq, Bq, Ak, Bk, Av, Bv, W_out, b_out):
    """Build the per-core input maps (shard x over batch; params replicated)."""
    f = lambda a: np.ascontiguousarray(a, dtype=np.float32)
    common = {
        "w_inT": f(W_in.T),
        "aqT": f(Aq.T),
        "akT": f(Ak.T),
        "avT": f(Av.T),
        "b_in_bc": f(np.broadcast_to(b_in, (B, H))),
        "bq_bc": f(np.broadcast_to(Bq, (B, H))),
        "bk_bc": f(np.broadcast_to(Bk, (B, H))),
        "bv_bc": f(np.broadcast_to(Bv, (B, H))),
        "w_outT": f(W_out.T),
        "b_out_bc": f(np.broadcast_to(b_out, (B, O))),
        "eye64": f(np.eye(B)),
    }
    xs = f(x).reshape(N_CORES, B, IN)
    return [dict(common, xT=np.ascontiguousarray(xs[i].T)) for i in range(N_CORES)]


def _get_nc():
    if "nc" not in _cache:
        _cache["nc"] = _build_nc()
    return _cache["nc"]


def run_spmd(in_maps, trace=False):
    from concourse.bass_utils import run_bass_kernel_spmd

    nc = _get_nc()
    res = run_bass_kernel_spmd(nc, in_maps, core_ids=list(range(N_CORES)), trace=trace)
    return res


def kernel(x, na=None, W_in=None, b_in=None, Aq=None, Bq=None, Ak=None, Bk=None,
           Av=None, Bv=None, W_out=None, b_out=None):
    in_maps = _host_inputs(x, W_in, b_in, Aq, Bq, Ak, Bk, Av, Bv, W_out, b_out)
    res = run_spmd(in_maps)
    out = np.concatenate([r["out"] for r in res.results], axis=0)
    return out.astype(np.float32)

